# revision 37
# baseline (speedup 1.0000x reference)
"""Trainium2 Bass kernel for nn_Correct_PrototypeManager (segment_reduce).

Reference computation:
    pred_lbl = argmax(preds, axis=1)                      # [B, H, W]
    feats_up = bilinear_resize(feats, H, W)               # [B, C, H, W]
    joint[b,k,h,w] = (masks==k) & (pred_lbl==k)
    counts[b,k] = sum_hw joint ; sums[b,k,c] = sum_hw feats_up * joint
    proto = mean_b( sums / (counts + eps) )               # [K, C]

Algebraic transform: bilinear upsample is linear, feats_up = (Uh (x) Uw) @
feats, so sums[k,c] = <U^T joint_k, feats_c>: downsample the one-hot joint
map (256^2 -> 64^2) with the adjoint of the upsample and contract over 4096
coarse pixels. Counts are preserved exactly (rows of U sum to 1).

V2 design (from the V1 66us trace):
  - DVE is the bottleneck: ~33us of irreducible work (tree f32 1x, eq f32
    1x, mult f16 2x, one-hot f16 4x). Everything else is scheduled around
    keeping the DVE stream dense from ~4.5us (post-barrier) onward.
  - ALL input DMAs ride the two HWDGE rings (sync + scalar): SWDGE (gpsimd)
    cost the V1 kernel ~14us of DVE idle (mask landed at 17.9us). Now:
    sync ring: mask first (131KB, ~0.4us), then the 4 preds k-chunks;
    scalar ring: u16/ucat consts then feats. One-hot starts at ~4.5us and
    the tree chases the preds chunks as they land.
  - eq/mult are emitted in k-granules matched to the preds chunks; the last
    h1 granule is smallest so the PE tail starts ASAP.
  - per-half PE pipeline: stage1 (joint-stationary, pre-transposed), stage2
    (ucat [U|U] stationary -> psb -> bsh via ACT copies), then the final
    contraction chases bsh hc-chunks (8 q-chunk matmuls per bsh chunk),
    accumulating both halves into one psf [K, C+1] PSUM. The whole h0 chain
    runs inside the DVE h1 window; only the h1 chain tails out (~6us).
  - fp16 everywhere downstream of eq, exact: U multiples of 1/8, A <= ~4,
    ds <= ~16.25 on representable grids; PE accumulates in f32 PSUM. Only
    feats fp16 rounding (~5e-5 rel) leaks into the result.

Sharding: data-parallel over batch B=8, one image per NeuronCore; the
[K, C+1] per-image partials (sums | counts) are combined on host.
"""

import numpy as np

B = 8
C = 256
K = 21
HC = WC = 64
HF = WF = 256
EPS = 1e-6
N_CORES = 8
PIX = HC * WC  # 4096
KA = 11        # preds k-chunk split: classes [0,KA) then [KA,K)

_PROGRAM_CACHE: dict = {}


def _upsample_matrix(n_in: int, n_out: int) -> np.ndarray:
    """U [n_out, n_in] with resize(x, 'bilinear', half-pixel) == U @ x."""
    U = np.zeros((n_out, n_in), dtype=np.float64)
    scale = n_in / n_out
    for i in range(n_out):
        src = (i + 0.5) * scale - 0.5
        f = int(np.floor(src))
        w = src - f
        lo = min(max(f, 0), n_in - 1)
        hi = min(max(f + 1, 0), n_in - 1)
        U[i, lo] += 1.0 - w
        U[i, hi] += w
    return U.astype(np.float32)


def _build_program():
    import concourse.bass as bass
    import concourse.bacc as bacc
    import concourse.tile as tile
    from concourse import mybir
    from contextlib import ExitStack

    f16 = mybir.dt.float16
    f32 = mybir.dt.float32
    OP = mybir.AluOpType

    nc = bacc.Bacc("TRN2", target_bir_lowering=False, debug=False,
                   num_devices=N_CORES)

    preds_d = nc.dram_tensor("preds", [2, 128, K, WF], f32,
                             kind="ExternalInput")
    mask_d = nc.dram_tensor("mask", [128, 2, WF], f16, kind="ExternalInput")
    feats_d = nc.dram_tensor("feats", [128, 32, C + 1], f16,
                             kind="ExternalInput")
    u_d = nc.dram_tensor("u", [2, 128, HC], f16, kind="ExternalInput")
    ucat_d = nc.dram_tensor("ucat", [2, 128, 128], f16, kind="ExternalInput")
    out_d = nc.dram_tensor("out", [K, C + 1], f32, kind="ExternalOutput")

    with tile.TileContext(nc) as tc, ExitStack() as ctx:
        const_pool = ctx.enter_context(tc.tile_pool(name="const", bufs=1))
        data_pool = ctx.enter_context(tc.tile_pool(name="data", bufs=1))
        res_pool = ctx.enter_context(tc.tile_pool(name="res", bufs=1))
        pst_pool = ctx.enter_context(
            tc.tile_pool(name="pst", bufs=2, space="PSUM"))
        psb_pool = ctx.enter_context(
            tc.tile_pool(name="psb", bufs=4, space="PSUM"))
        psf_pool = ctx.enter_context(
            tc.tile_pool(name="psf", bufs=1, space="PSUM"))
        psw_pool = ctx.enter_context(
            tc.tile_pool(name="psw", bufs=1, space="PSUM"))

        # ---- input DMAs, all HWDGE. scalar ring: mask first (gates the
        # one-hot; tiny), then consts. sync ring: the 4 preds k-chunk DMAs
        # the tree chases, then feats (only needed by the final
        # contraction — kept off the early window so it doesn't steal
        # SDMA bandwidth from the preds chunks). ----
        # mask rides the SYNC ring first: the scalar/ACT ring issues its
        # first DMA ~6us later (ACT preamble includes a table load), which
        # would stall the one-hot.
        mask_t = data_pool.tile([128, 2 * WF], f16, tag="mask")
        nc.sync.dma_start(mask_t[:], mask_d.ap())

        preds_t = []
        for h in range(2):
            t = data_pool.tile([128, K * WF], f32, tag=f"preds{h}")
            preds_t.append(t)
        pvs = [preds_t[h][:].rearrange("p (k w) -> p k w", k=K)
               for h in range(2)]
        # 3 chunks per half (8, 8, 5 classes): finer DVE chase granularity
        for h in range(2):
            for (c0, c1) in ((0, 8), (8, 16), (16, K)):
                nc.sync.dma_start(pvs[h][:, c0:c1, :],
                                  preds_d.ap()[h][:, c0:c1, :])
        ft_big = data_pool.tile([128, 32 * (C + 1)], f16, tag="ftbig")
        nc.sync.dma_start(ft_big[:], feats_d.ap())

        u16_t = []
        ucat_t = []
        for h in range(2):
            t = const_pool.tile([128, HC], f16, tag=f"u16_{h}")
            nc.scalar.dma_start(t[:], u_d.ap()[h])
            u16_t.append(t)
        for h in range(2):
            t = const_pool.tile([128, 128], f16, tag=f"ucat_{h}")
            nc.scalar.dma_start(t[:], ucat_d.ap()[h])
            ucat_t.append(t)

        # ---- one-hot of mask: oh4[p, k, h2, wf] via tensor_scalar (4x).
        # Split 14 + 7: part 1 fills the pre-chunk shadow, part 2 fills
        # the first inter-chunk DMA gap (emitted later). ----
        oh4 = data_pool.tile([128, K * 2 * WF], f16, tag="oh4")
        ohv = oh4[:].rearrange("p (k x) -> p k x", k=K)

        def _onehot(k0, k1):
            for k in range(k0, k1):
                nc.vector.tensor_scalar(
                    ohv[:, k, :], mask_t[:], float(k), None, OP.is_equal)
        ohv4 = oh4[:].rearrange("p (k h w) -> p k h w", k=K, h=2)

        # ---- per-half f32 max over classes: TT-max tree chasing the
        # THREE preds DMA k-chunks (8, 8, 5); temps shared across halves.
        # Emitted as three parts so each starts on its chunk arrival. ----
        t4a = data_pool.tile([128, 4 * WF], f32, tag="t4a")
        v4a = t4a[:].rearrange("p (k w) -> p k w", k=4)
        t4b = data_pool.tile([128, 4 * WF], f32, tag="t4b")
        v4b = t4b[:].rearrange("p (k w) -> p k w", k=4)
        t4d = data_pool.tile([128, 4 * WF], f32, tag="t4d")
        v4d = t4d[:].rearrange("p (k w) -> p k w", k=4)
        t2 = data_pool.tile([128, 2 * WF], f32, tag="t2")
        v2 = t2[:].rearrange("p (k w) -> p k w", k=2)
        t2e = data_pool.tile([128, 2 * WF], f32, tag="t2e")
        v2e = t2e[:].rearrange("p (k w) -> p k w", k=2)
        m1 = data_pool.tile([128, WF], f32, tag="m1")
        m2 = data_pool.tile([128, WF], f32, tag="m2")
        mA = data_pool.tile([128, WF], f32, tag="mA")
        maxv_t = []
        for h in range(2):
            mx = data_pool.tile([128, WF], f32, tag=f"maxv_{h}")
            maxv_t.append(mx)

        def _tree_a(h):
            nc.vector.tensor_tensor(v4a, pvs[h][:, 0:4, :], pvs[h][:, 4:8, :],
                                    op=OP.max)

        def _tree_b(h):
            dve = nc.vector
            dve.tensor_tensor(v4b, pvs[h][:, 8:12, :], pvs[h][:, 12:16, :],
                              op=OP.max)
            dve.tensor_tensor(v4d, v4a, v4b, op=OP.max)
            dve.tensor_tensor(v2e, v4d[:, 0:2, :], v4d[:, 2:4, :], op=OP.max)
            dve.tensor_tensor(m1[:], v2e[:, 0, :], v2e[:, 1, :], op=OP.max)

        def _tree_c(h):
            dve = nc.vector
            dve.tensor_tensor(v2, pvs[h][:, 16:18, :], pvs[h][:, 18:20, :],
                              op=OP.max)
            dve.tensor_tensor(m2[:], v2[:, 0, :], v2[:, 1, :], op=OP.max)
            dve.tensor_tensor(mA[:], m2[:], pvs[h][:, 20, :], op=OP.max)
            dve.tensor_tensor(maxv_t[h][:], m1[:], mA[:], op=OP.max)

        eq_t = []
        joint_t = []
        for h in range(2):
            eqh = data_pool.tile([128, K * WF], f16, tag=f"eq{h}")
            eq_t.append(eqh)
            jh = data_pool.tile([128, K * WF], f16, tag=f"joint{h}")
            joint_t.append(jh)
        eqv = [eq_t[h][:].rearrange("p (k w) -> p k w", k=K) for h in range(2)]
        jv = [joint_t[h][:].rearrange("p (k w) -> p k w", k=K)
              for h in range(2)]

        def _eq(h, g0, g1):
            ng = g1 - g0
            bc = (maxv_t[h][:].unsqueeze(1).to_broadcast([128, ng, WF]))
            nc.vector.tensor_tensor(
                eqv[h][:, g0:g1, :], pvs[h][:, g0:g1, :], bc, op=OP.is_ge)

        def _mul(h, g0, g1):
            nc.vector.tensor_tensor(
                jv[h][:, g0:g1, :], eqv[h][:, g0:g1, :],
                ohv4[:, g0:g1, h, :], op=OP.mult)

        # DVE stream: onehot, tree h0, eq h0, mult h0, tree h1, eq h1,
        # mult h1. The last h1 granule is a single class so the PE tail
        # entry after the final DVE op is minimal.
        GRAN = {0: ((0, 8), (8, 16), (16, 21)),
                1: ((0, 8), (8, 16), (16, 20), (20, 21))}

        # ---- stage 1, PRE-TRANSPOSED: at_unit[wf, hc] = joint_chunk^T @ Uh
        # (joint stationary -> output lands wf-on-partitions). One PSUM tile
        # gathers 8 units (4 classes); ACT copies it to at_h[wf, k, v, hc]
        # with the fp16 cast. ----
        at_big = []
        atv = []
        for h in range(2):
            ath = data_pool.tile([128, K * 2 * HC], f16, tag=f"at{h}")
            at_big.append(ath)
            atv.append(ath[:].rearrange("p (k v h) -> p k v h", k=K, v=2))

        def _stage1(h, k0, k1, copy_on_dve=False):
            # pst = up to 4 classes (1 PSUM bank); the copies are the
            # tail-entry critical path, so the last granule's copies can
            # ride the DVE (idle after its final mult)
            for kb in range(k0, k1, 4):
                ng = min(4, k1 - kb)
                pst = pst_pool.tile([128, 512], f32, tag="pst")
                for u in range(2 * ng):
                    k = kb + u // 2
                    wh = u % 2
                    nc.tensor.matmul(
                        pst[:, u * 64:(u + 1) * 64],
                        joint_t[h][:, k * WF + wh * 128:
                                   k * WF + wh * 128 + 128],
                        u16_t[h][:, :],
                        start=True, stop=True)
                eng = nc.vector.tensor_copy if copy_on_dve else nc.scalar.copy
                eng(atv[h][:, kb:kb + ng, :, :],
                    pst[:, :ng * 128].rearrange("p (n v h) -> p n v h",
                                                n=ng, v=2))

        # ---- stage 2 per half: psb[wc(dbl), k, hc17] accumulates the two
        # wf-halves via the [U|U] stationary; ACT copies the lo/hi hc rows
        # into b_sh with the odd-hc shift. Then the final contraction
        # chases each bsh hc-chunk: q-chunks 8c..8c+8 use bsh cols
        # [16c, 16c+16). psf [K, C+1] accumulates across both halves. ----
        b_sh = []
        for h in range(2):
            t = data_pool.tile([128, HC * K], f16, tag=f"bsh{h}")
            b_sh.append(t)
        bshv = [b_sh[h][:].rearrange("p (h k) -> p h k", h=HC)
                for h in range(2)]
        atm = [at_big[h][:].rearrange("p (k v h) -> p v k h", k=K, v=2)
               for h in range(2)]
        ftv = ft_big[:].rearrange("p (x c) -> p x c", x=32)
        psf = psf_pool.tile([K, C + 1], f32, tag="fin")

        def _final_chunk(h, c):
            # 8 q-chunk matmuls over bsh hc-range [16c, 16c+16); the 257th
            # feats column is 1.0 so column C lands counts[k]
            for ch in range(8 * c, 8 * c + 8):
                nc.tensor.matmul(
                    psf[:, :],
                    bshv[h][:, 2 * ch, :],
                    ftv[:, ch, :],
                    start=(h == 0 and ch == 0),
                    stop=(h == 1 and ch == 31))

        def _psb_alloc():
            pbvs = []
            for c in range(4):
                psb = psb_pool.tile([128, K * 17], f32, tag="psb")
                pbvs.append(psb[:].rearrange("p (k h) -> p k h", h=17))
            return pbvs

        def _psb_mms(h, pbvs, g0, g1):
            # accumulate the k-granule [g0,g1) of all 4 hc-chunks; each
            # k-slice of psb is its own 2-MM (v0 start, v1 stop) group
            for v in range(2):
                for c in range(4):
                    nh = 17 if c < 3 else 16
                    nc.tensor.matmul(
                        pbvs[c][:, g0:g1, :nh], ucat_t[v][:, :],
                        atm[h][:, v, g0:g1, 16 * c:16 * c + nh],
                        start=(v == 0), stop=(v == 1),
                        skip_group_check=True)

        def _psb_copies(h, pbvs):
            for c in range(4):
                pbv = pbvs[c]
                h0 = 16 * c
                nc.scalar.copy(bshv[h][0:64, h0:h0 + 16, :],
                               pbv[0:64, :, 0:16].transpose([0, 2, 1]))
                nhi = 16 if c < 3 else 15
                hi_eng = (nc.vector.tensor_copy if h == 1
                          else nc.scalar.copy)
                hi_eng(bshv[h][64:128, h0:h0 + nhi, :],
                       pbv[64:128, :, 1:1 + nhi].transpose([0, 2, 1]))

        # A 1-MM dummy gated on a DVE-written slice BLOCKS everything
        # behind it in the in-order PE queue: used to SPREAD the (already
        # data-ready) h0 final groups across the DVE h1 window so the PE
        # never idles a full HAM MID window and the tail runs at 2.4GHz.
        def _gate(stat, mov):
            t = psw_pool.tile([128, 256], f32, tag="warm")
            nc.tensor.matmul(t[:, :mov.shape[1]], stat, mov,
                             start=True, stop=True)

        # ---- emit in intended per-engine execution order ----
        _onehot(0, 14)       # pre-chunk DMA shadow
        _tree_a(0)
        _onehot(14, 21)      # fills the chunkA->chunkB DMA gap
        _tree_b(0)
        _tree_c(0)
        _eq(0, 0, 11)
        _eq(0, 11, 21)
        for (g0, g1) in GRAN[0]:
            _mul(0, g0, g1)
            _stage1(0, g0, g1)
        _tree_a(1)
        _tree_b(1)
        _tree_c(1)
        pbv0 = _psb_alloc()
        for (g0, g1) in GRAN[0]:
            _psb_mms(0, pbv0, g0, g1)
        _psb_copies(0, pbv0)
        _final_chunk(0, 0)
        _final_chunk(0, 1)
        _eq(1, 0, 11)
        _final_chunk(0, 2)
        _eq(1, 11, 21)
        _final_chunk(0, 3)
        pbv1 = _psb_alloc()
        for (g0, g1) in GRAN[1]:
            _mul(1, g0, g1)
            _stage1(1, g0, g1, copy_on_dve=(g0 >= 16))
            _psb_mms(1, pbv1, g0, g1)
        _psb_copies(1, pbv1)
        for c in range(4):
            _final_chunk(1, c)

        res_t = res_pool.tile([K, C + 1], f32, tag="res")
        nc.scalar.copy(res_t[:], psf[:])
        nc.sync.dma_start(out_d.ap()[:, :], res_t[:])

    nc.compile()
    return nc


def _get_program():
    if "nc" not in _PROGRAM_CACHE:
        _PROGRAM_CACHE["nc"] = _build_program()
    return _PROGRAM_CACHE["nc"]


def _host_inputs(feats, preds, masks):
    U = _upsample_matrix(HC, HF)  # [256, 64] f32, entries multiples of 1/8
    u16 = U.reshape(2, 128, HC).astype(np.float16)
    ucat = np.concatenate([u16, u16], axis=2)  # [2, 128, 128]

    # layouts: every device tensor is contiguous per SBUF partition
    preds32 = np.asarray(preds, dtype=np.float32)  # [B, K, 256, 256]
    predsx = np.ascontiguousarray(
        preds32.reshape(B, K, 2, 128, WF).transpose(0, 2, 3, 1, 4))
    mask16 = np.ascontiguousarray(
        np.asarray(masks).astype(np.float16)
        .reshape(B, 2, 128, WF).transpose(0, 2, 1, 3))
    feats32 = np.asarray(feats, dtype=np.float32).reshape(B, C, PIX)

    in_maps = []
    for b in range(B):
        ft = np.empty((PIX, C + 1), dtype=np.float16)
        ft[:, :C] = feats32[b].T
        ft[:, C] = 1.0  # ones column -> counts fall out of the final matmul
        ftx = np.ascontiguousarray(
            ft.reshape(32, 128, C + 1).transpose(1, 0, 2))
        in_maps.append({
            "preds": predsx[b],
            "mask": mask16[b],
            "feats": ftx,
            "u": u16,
            "ucat": ucat,
        })
    return in_maps


def kernel(feats, preds, masks, _results_hook=None):
    from concourse.bass_utils import run_bass_kernel_spmd

    nc = _get_program()
    in_maps = _host_inputs(feats, preds, masks)
    res = run_bass_kernel_spmd(nc, in_maps, list(range(N_CORES)))
    if _results_hook is not None:
        _results_hook(res)

    protos = []
    for b in range(B):
        out = res.results[b]["out"]  # [K, C+1] f32
        sums = out[:, :C]            # [K, C]
        counts = out[:, C]           # [K]
        protos.append(sums / (counts + EPS)[:, None])  # [K, C]
    return np.mean(np.stack(protos), axis=0).astype(np.float32)
